# revision 2
# baseline (speedup 1.0000x reference)
"""Trainium2 Bass kernel for CompoundMultivariateEmbedding (v2).

Math: out[n] = concat(level_tab[l], type_tab[t], feat_tab[f], exch_tab[e],
pair_tab[p]) @ W.T + b.  Because W is applied to a concat of block lookups,
out[n] = sum_b Ptab_b[idx_b[n]] + b where Ptab_b = tab_b @ W[:, block_b].T.
We place the projected tables in P [92, 128] fp16 (vocab rows padded so the
level block sits in PE row-group 0-1 and the small blocks in row-group 2)
and compute out.T = P.T @ onehot(idx) on the PE with P stationary.

Per-core loop (tokens sharded 8 ways, 131072 tokens/core):
  1. idx int32 -> fp16 via SWDGE cast-DMA into partitions 96-100
  2. broadcast matmul (e_sel stationary at PE rows 96+, tile_position=(96,0))
     replicates the 5 idx rows to 92 vocab partitions in PSUM
  3. DVE tensor_scalar is_equal vs per-partition iota -> multi-hot st fp16
     (every Z_PERIOD-th supertile goes ACT Identity+bias then DVE is_equal
     at 4x to balance DVE/ACT load)
  4. two matmuls per 512 tokens accumulate P.T @ st into PSUM [128, 512]:
     small-tables block (K=28, rows 64-95) then level block (K=50, rows
     0-63) - disjoint PE row groups so they stream concurrently with the
     broadcast
  5. ACT copies PSUM fp32 -> SBUF fp16; HWDGE stores 1 MiB chunks to a
     transposed y [128, n_core] fp16; the host transposes/upcasts.
"""

import sys

sys.path.insert(0, "/opt/trn_rl_repo")

import numpy as np

import concourse.bass as bass
import concourse.tile as tile
from concourse import bacc, mybir
from concourse._compat import with_exitstack
from contextlib import ExitStack

F32 = mybir.dt.float32
F16 = mybir.dt.float16
I32 = mybir.dt.int32

N_FULL = 1048576
N_CORES = 8
EMBED = 128

TAB_NAMES = ["level_tab", "type_tab", "feature_tab", "exchange_tab", "pair_tab"]
IDX_NAMES = ["level_idx", "type_idx", "feature_idx", "exchange_idx", "pair_idx"]
TAB_ROWS = [50, 2, 2, 3, 20]
TAB_ATTR = [25, 25, 25, 25, 28]
# vocab rows: level 0-49, (pad 50-63), type 64-65, feat 66-67, exch 68-70,
# pair 71-90, bias 91.  Level block = PE rows 0-63 after round-up, small
# blocks + bias = PE rows 64-95, broadcast selector = rows 96-127.
VOFF = [0, 64, 66, 68, 71]
FOFF = [0, 25, 50, 75, 100]  # feature (W column) offset per block
V = 92
BIAS_ROW = 91
SMALL0 = 64  # first vocab row of the small-tables block

ST = 1024  # tokens per supertile (2 PSUM banks for psbc, 2 for pso)
OSB = 4096  # tokens per output store (1 MiB fp16)
FB = 16384  # tokens per idx DMA batch
Z_PERIOD = 9  # every Z_PERIOD-th supertile uses ACT Identity + DVE 4x path


@with_exitstack
def _emb_kernel(ctx, tc, y_ap, tabs, w_ap, b_ap, idxs, n_core):
    nc = tc.nc

    const = ctx.enter_context(tc.tile_pool(name="const", bufs=1))

    # ---- index helpers ----
    pidx = const.tile([128, 1], I32)
    nc.gpsimd.iota(pidx, pattern=[[0, 1]], base=0, channel_multiplier=1)
    pidx_f = const.tile([128, 1], F32)
    nc.vector.tensor_copy(pidx_f, pidx)
    iotaf = const.tile([128, 128], I32)
    nc.gpsimd.iota(iotaf, pattern=[[1, 128]], base=0, channel_multiplier=0)
    ident = const.tile([128, 128], F32)
    nc.vector.tensor_scalar(ident, iotaf, pidx_f[:, :], None, mybir.AluOpType.is_equal)

    # ---- setup: projected tables (PSUM pool closed before the main loop) ----
    setup = ExitStack()
    psum_set = setup.enter_context(
        tc.tile_pool(name="psum_set", bufs=1, space=bass.MemorySpace.PSUM)
    )

    # W^T
    w_sb = const.tile([128, 128], F32)
    nc.sync.dma_start(w_sb, w_ap)
    psum_wt = psum_set.tile([128, 128], F32, tag="pset")
    nc.tensor.transpose(psum_wt, w_sb, ident)
    wt_sb = const.tile([128, 128], F32)
    nc.scalar.copy(wt_sb, psum_wt)

    # projected tables -> pf32 [92, 128] (row 91 = bias, rows 50-63 zero)
    pf32 = const.tile([V, EMBED], F32)
    nc.vector.memset(pf32, 0.0)
    for j in range(5):
        rows, attr = TAB_ROWS[j], TAB_ATTR[j]
        tab_sb = const.tile([rows, attr], F32, name=f"tab{j}")
        nc.sync.dma_start(tab_sb, tabs[j])
        psum_tt = psum_set.tile([attr, rows], F32, tag="pset", name=f"ptt{j}")
        nc.tensor.transpose(psum_tt, tab_sb, ident[0:rows, 0:rows])
        tabt_sb = const.tile([attr, rows], F32, name=f"tabt{j}")
        nc.scalar.copy(tabt_sb, psum_tt)
        wb_sb = const.tile([attr, EMBED], F32, name=f"wb{j}")
        nc.gpsimd.dma_start(wb_sb, wt_sb[FOFF[j] : FOFF[j] + attr, :])
        psum_pb = psum_set.tile([rows, EMBED], F32, tag="pset", name=f"ppb{j}")
        nc.tensor.matmul(psum_pb, tabt_sb, wb_sb)
        pb_sb = const.tile([rows, EMBED], F32, name=f"pb{j}")
        nc.scalar.copy(pb_sb, psum_pb)
        nc.gpsimd.dma_start(pf32[VOFF[j] : VOFF[j] + rows, :], pb_sb)
    nc.sync.dma_start(pf32[BIAS_ROW : BIAS_ROW + 1, :], b_ap)

    pf16 = const.tile([V, EMBED], F16)
    nc.vector.tensor_copy(pf16, pf32)

    setup.close()  # free setup PSUM banks

    # ---- E selector [5, 92] at partitions 96-100 ----
    e_row = const.tile([1, 5 * V], F16)
    nc.vector.memset(e_row, 0.0)
    for j in range(5):
        nc.vector.memset(
            e_row[:, j * V + VOFF[j] : j * V + VOFF[j] + TAB_ROWS[j]], 1.0
        )
    e_sel = const.tile([128, V], F16)
    nc.gpsimd.dma_start(e_sel[96:101, :], e_row)

    # ---- iota column: within-block index per vocab partition ----
    off_row = const.tile([1, 128], F32)
    nc.vector.memset(off_row, 10000.0)  # pad rows never match
    for j in range(5):
        nc.vector.memset(off_row[:, VOFF[j] : VOFF[j] + TAB_ROWS[j]], float(VOFF[j]))
    nc.vector.memset(off_row[:, BIAS_ROW : BIAS_ROW + 1], float(BIAS_ROW))
    offc = const.tile([128, 1], F32)
    nc.gpsimd.dma_start(offc, off_row)
    iota_col = const.tile([128, 1], F32)
    nc.vector.tensor_sub(iota_col, pidx_f, offc)
    niota_col = const.tile([128, 1], F32)
    nc.vector.tensor_sub(niota_col, offc, pidx_f)

    # ---- main loop ----
    idx_pool = ctx.enter_context(tc.tile_pool(name="idxp", bufs=2))
    st_pool = ctx.enter_context(tc.tile_pool(name="stp", bufs=4))
    out_pool = ctx.enter_context(tc.tile_pool(name="outp", bufs=2))
    psbc_pool = ctx.enter_context(
        tc.tile_pool(name="pbc", bufs=2, space=bass.MemorySpace.PSUM)
    )
    pso_pool = ctx.enter_context(
        tc.tile_pool(name="pout", bufs=2, space=bass.MemorySpace.PSUM)
    )

    n_super = 0
    for bi in range(n_core // FB):
        idxf = idx_pool.tile([128, FB], F16)
        for j in range(5):
            # SWDGE cast-DMA: int32 -> fp16 in flight
            nc.gpsimd.dma_start(
                idxf[96 + j : 97 + j, :], idxs[j][bi * FB : (bi + 1) * FB]
            )
        for g in range(FB // OSB):
            osb = out_pool.tile([128, OSB], F16)
            for s in range(OSB // ST):
                k0 = g * OSB + s * ST
                psbc = psbc_pool.tile([V, ST], F32)
                for b in range(ST // 512):
                    nc.tensor.matmul(
                        psbc[:, bass.ts(b, 512)],
                        e_sel[96:101, :],
                        idxf[96:101, k0 + b * 512 : k0 + (b + 1) * 512],
                        tile_position=(96, 0),
                    )
                st = st_pool.tile([V, ST], F16)
                n_super += 1
                if Z_PERIOD and n_super % Z_PERIOD == 0:
                    # balance: ACT does the iota subtract, DVE finishes at 4x
                    yb = st_pool.tile([V, ST], F16, name="yb")
                    nc.scalar.activation(
                        yb,
                        psbc,
                        mybir.ActivationFunctionType.Identity,
                        bias=niota_col[0:V, :],
                    )
                    nc.vector.tensor_scalar(
                        st, yb, 0.0, None, mybir.AluOpType.is_equal
                    )
                else:
                    nc.vector.tensor_scalar(
                        st, psbc, iota_col[0:V, :], None, mybir.AluOpType.is_equal
                    )
                pso = pso_pool.tile([128, ST], F32)
                for b in range(ST // 512):
                    sl = bass.ts(b, 512)
                    # small blocks first (shorter PE pipe -> lands first),
                    # level accumulates on top; disjoint row groups stream
                    # concurrently with each other and the broadcast.
                    nc.tensor.matmul(
                        pso[:, sl],
                        pf16[SMALL0:V, :],
                        st[SMALL0:V, sl],
                        start=True,
                        stop=False,
                    )
                    nc.tensor.matmul(
                        pso[:, sl],
                        pf16[0:50, :],
                        st[0:50, sl],
                        start=False,
                        stop=True,
                    )
                nc.scalar.copy(osb[:, s * ST : (s + 1) * ST], pso)
            n0 = bi * FB + g * OSB
            nc.sync.dma_start(y_ap[:, n0 : n0 + OSB], osb)


def build(n_core, num_devices=N_CORES):
    nc = bacc.Bacc(
        "TRN2", target_bir_lowering=False, debug=False, num_devices=num_devices
    )
    tabs, idxs = [], []
    for j, nm in enumerate(TAB_NAMES):
        tabs.append(nc.dram_tensor(nm, [TAB_ROWS[j], TAB_ATTR[j]], F32,
                                   kind="ExternalInput").ap())
    w_ap = nc.dram_tensor("W", [EMBED, EMBED], F32, kind="ExternalInput").ap()
    b_ap = nc.dram_tensor("b", [EMBED], F32, kind="ExternalInput").ap()
    for nm in IDX_NAMES:
        idxs.append(nc.dram_tensor(nm, [n_core], I32, kind="ExternalInput").ap())
    # transposed output: [embed, tokens] fp16; host transposes + upcasts
    y = nc.dram_tensor("y", [EMBED, n_core], F16, kind="ExternalOutput")

    with tile.TileContext(nc) as tc:
        _emb_kernel(tc, y.ap(), tabs, w_ap, b_ap, idxs, n_core)
    nc.compile()
    return nc


_NC_CACHE = {}


def _get_nc(n_core):
    if n_core not in _NC_CACHE:
        _NC_CACHE[n_core] = build(n_core)
    return _NC_CACHE[n_core]


def _make_in_maps(inputs, n_cores, n_core):
    shared = {}
    for nm in TAB_NAMES + ["W", "b"]:
        shared[nm] = np.ascontiguousarray(np.asarray(inputs[nm], dtype=np.float32))
    in_maps = []
    for c in range(n_cores):
        m = dict(shared)
        for nm in IDX_NAMES:
            m[nm] = np.ascontiguousarray(
                np.asarray(inputs[nm], dtype=np.int32)[c * n_core : (c + 1) * n_core]
            )
        in_maps.append(m)
    return in_maps


TRACE_DIR = "/tmp/bass_trace"


def run(inputs, trace=False, tmpdir=None):
    """Run on hardware across 8 cores; returns (full_output, BassKernelResults)."""
    from concourse.bass_utils import run_bass_kernel_spmd

    n = np.asarray(inputs[IDX_NAMES[0]]).shape[0]
    n_core = n // N_CORES
    nc = _get_nc(n_core)
    in_maps = _make_in_maps(inputs, N_CORES, n_core)
    if trace and tmpdir is not None:
        import os
        import shutil

        shutil.rmtree(tmpdir, ignore_errors=True)
        os.makedirs(tmpdir, exist_ok=True)
    res = run_bass_kernel_spmd(nc, in_maps, core_ids=list(range(N_CORES)),
                               trace=trace, tmpdir=tmpdir if trace else None)
    out = np.empty((n, EMBED), dtype=np.float32)
    for c in range(N_CORES):
        out[c * n_core : (c + 1) * n_core, :] = res.results[c]["y"].T
    return out, res


def kernel(**inputs):
    out, _ = run(inputs)
    return out


# revision 3
# speedup vs baseline: 1.6005x; 1.6005x over previous
"""Trainium2 Bass kernel for CompoundMultivariateEmbedding (v2).

Math: out[n] = concat(level_tab[l], type_tab[t], feat_tab[f], exch_tab[e],
pair_tab[p]) @ W.T + b.  Because W is applied to a concat of block lookups,
out[n] = sum_b Ptab_b[idx_b[n]] + b where Ptab_b = tab_b @ W[:, block_b].T.
We place the projected tables in P [92, 128] fp16 (vocab rows padded so the
level block sits in PE row-group 0-1 and the small blocks in row-group 2)
and compute out.T = P.T @ onehot(idx) on the PE with P stationary.

Per-core loop (tokens sharded 8 ways, 131072 tokens/core):
  1. idx int32 -> fp16 via SWDGE cast-DMA into partitions 96-100
  2. broadcast matmul (e_sel stationary at PE rows 96+, tile_position=(96,0))
     replicates the 5 idx rows to 92 vocab partitions in PSUM
  3. DVE tensor_scalar is_equal vs per-partition iota -> multi-hot st fp16
     (every Z_PERIOD-th supertile goes ACT Identity+bias then DVE is_equal
     at 4x to balance DVE/ACT load)
  4. two matmuls per 512 tokens accumulate P.T @ st into PSUM [128, 512]:
     small-tables block (K=28, rows 64-95) then level block (K=50, rows
     0-63) - disjoint PE row groups so they stream concurrently with the
     broadcast
  5. ACT copies PSUM fp32 -> SBUF fp16; HWDGE stores 1 MiB chunks to a
     transposed y [128, n_core] fp16; the host transposes/upcasts.
"""

import sys

sys.path.insert(0, "/opt/trn_rl_repo")

import numpy as np

import concourse.bass as bass
import concourse.tile as tile
from concourse import bacc, mybir
from concourse._compat import with_exitstack
from contextlib import ExitStack

F32 = mybir.dt.float32
F16 = mybir.dt.float16
I32 = mybir.dt.int32

N_FULL = 1048576
N_CORES = 8
EMBED = 128

TAB_NAMES = ["level_tab", "type_tab", "feature_tab", "exchange_tab", "pair_tab"]
IDX_NAMES = ["level_idx", "type_idx", "feature_idx", "exchange_idx", "pair_idx"]
TAB_ROWS = [50, 2, 2, 3, 20]
TAB_ATTR = [25, 25, 25, 25, 28]
# vocab rows: level 0-49, (pad 50-63), type 64-65, feat 66-67, exch 68-70,
# pair 71-90, bias 91.  Level block = PE rows 0-63 after round-up, small
# blocks + bias = PE rows 64-95, broadcast selector = rows 96-127.
VOFF = [0, 64, 66, 68, 71]
FOFF = [0, 25, 50, 75, 100]  # feature (W column) offset per block
V = 92
BIAS_ROW = 91
SMALL0 = 64  # first vocab row of the small-tables block

ST = 1024  # tokens per supertile (2 PSUM banks for psbc, 2 for pso)
OSB = 4096  # tokens per output store (1 MiB fp16)
FB = 16384  # tokens per idx DMA batch
Z_PERIOD = 9  # every Z_PERIOD-th supertile uses ACT Identity + DVE 4x path


@with_exitstack
def _emb_kernel(ctx, tc, y_ap, tabs, w_ap, b_ap, idxs, n_core):
    nc = tc.nc

    const = ctx.enter_context(tc.tile_pool(name="const", bufs=1))

    # ---- index helpers ----
    pidx = const.tile([128, 1], I32)
    nc.gpsimd.iota(pidx, pattern=[[0, 1]], base=0, channel_multiplier=1)
    pidx_f = const.tile([128, 1], F32)
    nc.vector.tensor_copy(pidx_f, pidx)
    iotaf = const.tile([128, 128], I32)
    nc.gpsimd.iota(iotaf, pattern=[[1, 128]], base=0, channel_multiplier=0)
    ident = const.tile([128, 128], F32)
    nc.vector.tensor_scalar(ident, iotaf, pidx_f[:, :], None, mybir.AluOpType.is_equal)

    # ---- setup: projected tables (PSUM pool closed before the main loop) ----
    setup = ExitStack()
    psum_set = setup.enter_context(
        tc.tile_pool(name="psum_set", bufs=1, space=bass.MemorySpace.PSUM)
    )

    # W^T
    w_sb = const.tile([128, 128], F32)
    nc.sync.dma_start(w_sb, w_ap)
    psum_wt = psum_set.tile([128, 128], F32, tag="pset")
    nc.tensor.transpose(psum_wt, w_sb, ident)
    wt_sb = const.tile([128, 128], F32)
    nc.scalar.copy(wt_sb, psum_wt)

    # projected tables -> pf32 [92, 128] (row 91 = bias, rows 50-63 zero)
    pf32 = const.tile([V, EMBED], F32)
    nc.vector.memset(pf32, 0.0)
    for j in range(5):
        rows, attr = TAB_ROWS[j], TAB_ATTR[j]
        tab_sb = const.tile([rows, attr], F32, name=f"tab{j}")
        nc.sync.dma_start(tab_sb, tabs[j])
        psum_tt = psum_set.tile([attr, rows], F32, tag="pset", name=f"ptt{j}")
        nc.tensor.transpose(psum_tt, tab_sb, ident[0:rows, 0:rows])
        tabt_sb = const.tile([attr, rows], F32, name=f"tabt{j}")
        nc.scalar.copy(tabt_sb, psum_tt)
        wb_sb = const.tile([attr, EMBED], F32, name=f"wb{j}")
        nc.gpsimd.dma_start(wb_sb, wt_sb[FOFF[j] : FOFF[j] + attr, :])
        psum_pb = psum_set.tile([rows, EMBED], F32, tag="pset", name=f"ppb{j}")
        nc.tensor.matmul(psum_pb, tabt_sb, wb_sb)
        pb_sb = const.tile([rows, EMBED], F32, name=f"pb{j}")
        nc.scalar.copy(pb_sb, psum_pb)
        nc.gpsimd.dma_start(pf32[VOFF[j] : VOFF[j] + rows, :], pb_sb)
    nc.sync.dma_start(pf32[BIAS_ROW : BIAS_ROW + 1, :], b_ap)

    pf16 = const.tile([V, EMBED], F16)
    nc.vector.tensor_copy(pf16, pf32)

    setup.close()  # free setup PSUM banks

    # ---- E selector [5, 92] at partitions 96-100 ----
    e_row = const.tile([1, 5 * V], F16)
    nc.vector.memset(e_row, 0.0)
    for j in range(5):
        nc.vector.memset(
            e_row[:, j * V + VOFF[j] : j * V + VOFF[j] + TAB_ROWS[j]], 1.0
        )
    e_sel = const.tile([128, V], F16)
    nc.gpsimd.dma_start(e_sel[96:101, :], e_row)

    # ---- iota column: within-block index per vocab partition ----
    off_row = const.tile([1, 128], F32)
    nc.vector.memset(off_row, 10000.0)  # pad rows never match
    for j in range(5):
        nc.vector.memset(off_row[:, VOFF[j] : VOFF[j] + TAB_ROWS[j]], float(VOFF[j]))
    nc.vector.memset(off_row[:, BIAS_ROW : BIAS_ROW + 1], float(BIAS_ROW))
    offc = const.tile([128, 1], F32)
    nc.gpsimd.dma_start(offc, off_row)
    iota_col = const.tile([128, 1], F32)
    nc.vector.tensor_sub(iota_col, pidx_f, offc)
    niota_col = const.tile([128, 1], F32)
    nc.vector.tensor_sub(niota_col, offc, pidx_f)

    # ---- main loop (software-pipelined) ----
    # PE stream per supertile k: [bcast(k+1), main(k)] so the broadcast for
    # the next supertile fills the PE while DVE runs is_equal(k) - no PE
    # head-of-line stall, keeps the HAM clock-gate warm.
    idx_pool = ctx.enter_context(tc.tile_pool(name="idxp", bufs=2))
    st_pool = ctx.enter_context(tc.tile_pool(name="stp", bufs=4))
    out_pool = ctx.enter_context(tc.tile_pool(name="outp", bufs=2))
    psbc_pool = ctx.enter_context(
        tc.tile_pool(name="pbc", bufs=2, space=bass.MemorySpace.PSUM)
    )
    pso_pool = ctx.enter_context(
        tc.tile_pool(name="pout", bufs=2, space=bass.MemorySpace.PSUM)
    )

    n_sup = n_core // ST
    sup_per_osb = OSB // ST
    idxf_tiles = {}

    def load_idx_batch(b):
        t = idx_pool.tile([128, FB], F16, name=f"idxf{b % 2}")
        for j in range(5):
            # SWDGE cast-DMA: int32 -> fp16 in flight
            nc.gpsimd.dma_start(t[96 + j : 97 + j, :], idxs[j][b * FB : (b + 1) * FB])
        idxf_tiles[b] = t

    def bcast(k):
        b, off = divmod(k * ST, FB)
        psbc = psbc_pool.tile([V, ST], F32)
        for c in range(ST // 512):
            nc.tensor.matmul(
                psbc[:, bass.ts(c, 512)],
                e_sel[96:101, :],
                idxf_tiles[b][96:101, off + c * 512 : off + (c + 1) * 512],
                tile_position=(96, 0),
            )
        return psbc

    load_idx_batch(0)
    psbc_cur = bcast(0)
    osb = None
    for k in range(n_sup):
        # prefetch the idx batch two supertiles ahead
        b_ahead = ((k + 2) * ST) // FB
        if b_ahead < n_core // FB and b_ahead not in idxf_tiles:
            load_idx_batch(b_ahead)
        psbc_nxt = bcast(k + 1) if k + 1 < n_sup else None
        st = st_pool.tile([V, ST], F16)
        if Z_PERIOD and k % Z_PERIOD == Z_PERIOD - 1:
            # balance: ACT does the iota subtract, DVE finishes at 4x
            yb = st_pool.tile([V, ST], F16, name="yb")
            nc.scalar.activation(
                yb,
                psbc_cur,
                mybir.ActivationFunctionType.Identity,
                bias=niota_col[0:V, :],
            )
            nc.vector.tensor_scalar(st, yb, 0.0, None, mybir.AluOpType.is_equal)
        else:
            nc.vector.tensor_scalar(
                st, psbc_cur, iota_col[0:V, :], None, mybir.AluOpType.is_equal
            )
        pso = pso_pool.tile([128, ST], F32)
        for c in range(ST // 512):
            sl = bass.ts(c, 512)
            nc.tensor.matmul(pso[:, sl], pf16, st[:, sl])
        if k % sup_per_osb == 0:
            osb = out_pool.tile([128, OSB], F16)
        nc.scalar.copy(osb[:, (k % sup_per_osb) * ST : (k % sup_per_osb + 1) * ST], pso)
        if (k + 1) % sup_per_osb == 0:
            n0 = (k + 1 - sup_per_osb) * ST
            nc.sync.dma_start(y_ap[:, n0 : n0 + OSB], osb)
        psbc_cur = psbc_nxt


def build(n_core, num_devices=N_CORES):
    nc = bacc.Bacc(
        "TRN2", target_bir_lowering=False, debug=False, num_devices=num_devices
    )
    tabs, idxs = [], []
    for j, nm in enumerate(TAB_NAMES):
        tabs.append(nc.dram_tensor(nm, [TAB_ROWS[j], TAB_ATTR[j]], F32,
                                   kind="ExternalInput").ap())
    w_ap = nc.dram_tensor("W", [EMBED, EMBED], F32, kind="ExternalInput").ap()
    b_ap = nc.dram_tensor("b", [EMBED], F32, kind="ExternalInput").ap()
    for nm in IDX_NAMES:
        idxs.append(nc.dram_tensor(nm, [n_core], I32, kind="ExternalInput").ap())
    # transposed output: [embed, tokens] fp16; host transposes + upcasts
    y = nc.dram_tensor("y", [EMBED, n_core], F16, kind="ExternalOutput")

    with tile.TileContext(nc) as tc:
        _emb_kernel(tc, y.ap(), tabs, w_ap, b_ap, idxs, n_core)
    nc.compile()
    return nc


_NC_CACHE = {}


def _get_nc(n_core):
    if n_core not in _NC_CACHE:
        _NC_CACHE[n_core] = build(n_core)
    return _NC_CACHE[n_core]


def _make_in_maps(inputs, n_cores, n_core):
    shared = {}
    for nm in TAB_NAMES + ["W", "b"]:
        shared[nm] = np.ascontiguousarray(np.asarray(inputs[nm], dtype=np.float32))
    in_maps = []
    for c in range(n_cores):
        m = dict(shared)
        for nm in IDX_NAMES:
            m[nm] = np.ascontiguousarray(
                np.asarray(inputs[nm], dtype=np.int32)[c * n_core : (c + 1) * n_core]
            )
        in_maps.append(m)
    return in_maps


TRACE_DIR = "/tmp/bass_trace"


def run(inputs, trace=False, tmpdir=None):
    """Run on hardware across 8 cores; returns (full_output, BassKernelResults)."""
    from concourse.bass_utils import run_bass_kernel_spmd

    n = np.asarray(inputs[IDX_NAMES[0]]).shape[0]
    n_core = n // N_CORES
    nc = _get_nc(n_core)
    in_maps = _make_in_maps(inputs, N_CORES, n_core)
    if trace and tmpdir is not None:
        import os
        import shutil

        shutil.rmtree(tmpdir, ignore_errors=True)
        os.makedirs(tmpdir, exist_ok=True)
    res = run_bass_kernel_spmd(nc, in_maps, core_ids=list(range(N_CORES)),
                               trace=trace, tmpdir=tmpdir if trace else None)
    out = np.empty((n, EMBED), dtype=np.float32)
    for c in range(N_CORES):
        out[c * n_core : (c + 1) * n_core, :] = res.results[c]["y"].T
    return out, res


def kernel(**inputs):
    out, _ = run(inputs)
    return out
